# revision 38
# baseline (speedup 1.0000x reference)
"""Trainium2 Bass kernel for CustomFlashAttention (B=2, S=2048, D=2048, H=16).

Sharding over 8 NeuronCores: core c handles batch b=c//4 and head-group
hg=c%4 (4 heads of 128 dims = feature cols [hg*512,(hg+1)*512)).
Per core: QKV projections for its cols, causal flash attention for its 4
heads, partial output projection; host sums the 4 partials per batch.

v2 design (bf16): all matmul operands bf16 (fp32 PSUM accumulation),
which enables fast-weight-load and halves DMA/SBUF. Attention uses
natural-orientation PV with a ones-column appended to V so each PV
accumulation also produces the softmax row-sum in PSUM column 128 (no
separate ones-matmul). Per-q normalization is a per-partition
tensor_scalar multiply; the normalized O tile is transposed on the PE
(identity matmul) to feed the output projection. Scores stay in the
transposed [k, q] orientation with exact 128-granular causal trimming;
the diagonal 128x128 block is masked by a 0/1 triangular multiply after
exp (scores are bounded ~|s|<7 so unshifted exp is safe).

Schedule: pass A emits Q+K projections only (pure PE). Pass B emits V
projections with the full attention stream interleaved at matmul
granularity, pacing scores so the Activation engine's exp (the 2nd
busiest resource) always runs ahead of PV consumption. PSUM uses exactly
8 banks (tags qk/s/pv/po x2); the score/exp bf16 tile pool also recycles
the wq/wk weight buffers (same tag) to stay inside SBUF.
"""

import os
import numpy as np
import ml_dtypes

import concourse.bacc as bacc
import concourse.mybir as mybir
import concourse.tile as tile
from concourse.bass_utils import run_bass_kernel_spmd

B = 2
S = 2048
D = 2048
H_PER_CORE = 4
DC = 512          # feature cols per core (4 heads * 128)
HD = 128          # head dim
P = 128
TB = 512          # token block
N_TB = S // TB    # 4
N_KT = S // P     # 16 (128-wide k/token tiles)
FP32 = mybir.dt.float32
BF16 = mybir.dt.bfloat16
EXP = mybir.ActivationFunctionType.Exp
MULT = mybir.AluOpType.mult

PT_BUFS = 13      # rotation groups (4 kt-tiles each) for wq/wk + score tiles

LAST_RESULTS = None  # BassKernelResults from the most recent run (for test.py)


def build_bass(causal: bool):
    nc = bacc.Bacc(None, target_bir_lowering=False, debug=False)

    xT_d = nc.dram_tensor("xT", [D, S], BF16, kind="ExternalInput")
    wqT_d = nc.dram_tensor("wqT", [D, DC], BF16, kind="ExternalInput")
    wkT_d = nc.dram_tensor("wkT", [D, DC], BF16, kind="ExternalInput")
    wvT_d = nc.dram_tensor("wvT", [D, DC], BF16, kind="ExternalInput")
    woT_d = nc.dram_tensor("woT", [DC, D], BF16, kind="ExternalInput")
    tri_d = nc.dram_tensor("tri", [P, P], BF16, kind="ExternalInput")
    id_d = nc.dram_tensor("ident", [P, P], BF16, kind="ExternalInput")
    out_d = nc.dram_tensor("out", [S, D], FP32, kind="ExternalOutput")

    x_r = xT_d.rearrange("(ko p) t -> p ko t", p=P)     # [128, 16, 2048]
    wq_r = wqT_d.rearrange("(ko p) m -> p ko m", p=P)   # [128, 16, 512]
    wk_r = wkT_d.rearrange("(ko p) m -> p ko m", p=P)
    wv_r = wvT_d.rearrange("(ko p) m -> p ko m", p=P)
    wo_r = woT_d.rearrange("(h p) n -> p h n", p=P)     # [128, 4, 2048]

    with tile.TileContext(nc) as tc:
        with tc.tile_pool(name="persist", bufs=1) as persist, \
             tc.tile_pool(name="wp", bufs=PT_BUFS) as wp, \
             tc.tile_pool(name="xp", bufs=4) as xp, \
             tc.tile_pool(name="onp", bufs=16) as onp, \
             tc.tile_pool(name="rcp", bufs=4) as rcp, \
             tc.tile_pool(name="otp", bufs=2) as otp, \
             tc.tile_pool(name="obp", bufs=3) as obp, \
             tc.tile_pool(name="psqk", bufs=2, space="PSUM") as psqk, \
             tc.tile_pool(name="pss", bufs=2, space="PSUM") as pss, \
             tc.tile_pool(name="pspv", bufs=2, space="PSUM") as pspv, \
             tc.tile_pool(name="pso", bufs=2, space="PSUM") as pso:

            # ---- persistent tensors ----
            qt_s = persist.tile([P, H_PER_CORE, S], BF16)   # QT [d, h, tok]
            kt_s = persist.tile([P, H_PER_CORE, S], BF16)   # KT [d, h, tok]
            # V natural + ones column: [tok%128, h, tok//128, 129]
            v_s = persist.tile([P, H_PER_CORE, N_KT, HD + 1], BF16)
            wv_s = persist.tile([P, N_KT, DC], BF16)
            wo_s = persist.tile([P, H_PER_CORE, D], BF16)
            tri_s = persist.tile([P, P], BF16)
            id_s = persist.tile([P, P], BF16)

            # ---- weight/x prefetch. wq/wk live in 4-kt chunks that share
            # the rotation tag (and 4KB slot) with the score/exp tile
            # groups, so their SBUF space is recycled by the attention
            # stream and the whole prefetch is 8 DMA issues. ----
            wq_c = [wp.tile([P, 4, DC], BF16, tag="p", name=f"wq{c}")
                    for c in range(4)]
            wk_c = [wp.tile([P, 4, DC], BF16, tag="p", name=f"wk{c}")
                    for c in range(4)]

            xh = {}  # (tb, half) -> x tile [128 d, 8 kt, 512 tok]

            def dma_x(tb):
                for half in range(2):
                    t = xp.tile([P, 8, TB], BF16, tag="x",
                                name=f"x{tb}_{half}")
                    nc.sync.dma_start(
                        t[:], x_r[:, half * 8:half * 8 + 8,
                                  tb * TB:(tb + 1) * TB])
                    xh[(tb, half)] = t

            def xt(tb, kt):
                return xh[(tb, kt // 8)][:, kt % 8, :]

            # prefetch: the first 4 kt of x and wq land as small per-kt
            # DMAs (fast completion keeps pace with the cold-clock first
            # chain), the rest as chunks; the two DMA queues run in
            # parallel (x on sync, weights on scalar)
            xh0 = xp.tile([P, 8, TB], BF16, tag="x", name="x0_0")
            xh1 = xp.tile([P, 8, TB], BF16, tag="x", name="x0_1")
            xh[(0, 0)], xh[(0, 1)] = xh0, xh1
            for kt in range(4):
                wq_q = nc.scalar if kt % 2 == 0 else nc.sync
                x_q = nc.sync if kt % 2 == 0 else nc.scalar
                wq_q.dma_start(wq_c[0][:, kt, :], wq_r[:, kt, :])
                x_q.dma_start(xh0[:, kt, :], x_r[:, kt, 0:TB])
            nc.sync.dma_start(xh0[:, 4:8, :], x_r[:, 4:8, 0:TB])
            nc.sync.dma_start(xh1[:], x_r[:, 8:16, 0:TB])
            for c in range(1, 4):
                nc.scalar.dma_start(wq_c[c][:], wq_r[:, 4 * c:4 * c + 4, :])
            nc.sync.dma_start(wk_c[1][:], wk_r[:, 4:8, :])
            nc.sync.dma_start(wk_c[3][:], wk_r[:, 12:16, :])
            nc.scalar.dma_start(wk_c[0][:], wk_r[:, 0:4, :])
            nc.scalar.dma_start(wk_c[2][:], wk_r[:, 8:12, :])
            nc.scalar.dma_start(tri_s[:], tri_d[:])
            nc.scalar.dma_start(id_s[:], id_d[:])
            nc.scalar.dma_start(wv_s[:], wv_r[:])
            nc.scalar.dma_start(wo_s[:], wo_r[:])
            # ones columns of v_s (memset everything; V evicts overwrite data)
            nc.gpsimd.memset(v_s[:], 1.0)

            # warm-up matmuls on scratch data while the first DMAs land:
            # keeps the PE busy from ~2us so its clock is fully ramped
            # when the real projection chains start
            warm = persist.tile([P, TB], BF16, tag="warm")
            nc.vector.memset(warm[:], 0.0)
            for i in range(3):
                wps = pss.tile([P, TB], FP32, tag="s", name=f"warm{i}")
                nc.tensor.matmul(wps[:], warm[:, 0:P], warm[:],
                                 start=True, stop=True)

            # ---- pass A: Q and K projections (transposed layouts) ----
            for tb in range(N_TB):
                for which, w_c, dst in (("q", wq_c, qt_s), ("k", wk_c, kt_s)):
                    for h in range(H_PER_CORE):
                        ps = psqk.tile([P, TB], FP32, tag="qk",
                                       name=f"ps{which}_{tb}_{h}")
                        for kt in range(N_KT):
                            nc.tensor.matmul(
                                ps[:],
                                w_c[kt // 4][:, kt % 4,
                                             h * HD:(h + 1) * HD],
                                xt(tb, kt),
                                start=kt == 0, stop=kt == N_KT - 1)
                        nc.vector.tensor_copy(
                            out=dst[:, h, tb * TB:(tb + 1) * TB], in_=ps[:])
                        if which == "q" and h == 0 and tb + 1 < N_TB:
                            dma_x(tb + 1)

            # ---- pass B: V projections + interleaved attention ----
            n_kt_q = [4 * qb + 4 if causal else N_KT for qb in range(N_TB)]
            pt_tiles = {}
            pt_grp = {}

            # score/exp emission machinery, paced by a rotation window
            # counted in 4-kt tile groups (wq/wk occupy the first 8 slots)
            score_q = [(qb, h, kt)
                       for qb in range(N_TB)
                       for h in range(H_PER_CORE)
                       for kt in range(n_kt_q[qb])]
            state = {"emit": 0, "alloc": 8, "freed": 8, "credit": 0.0}

            def emit_score():
                qb, h, kt = score_q[state["emit"]]
                state["emit"] += 1
                delta = (kt - 4 * qb) * P if causal else -1
                s0 = max(0, min(delta, TB - P)) if causal else 0
                ps = pss.tile([P, TB], FP32, tag="s", name=f"s_{qb}_{h}_{kt}")
                nc.tensor.matmul(
                    ps[:, s0:],
                    kt_s[:, h, kt * P:(kt + 1) * P],
                    qt_s[:, h, qb * TB + s0:(qb + 1) * TB],
                    start=True, stop=True)
                if kt % 4 == 0:
                    state["alloc"] += 1
                    pt_grp[(qb, h, kt // 4)] = wp.tile(
                        [P, 4, TB], BF16, tag="p", name=f"p_{qb}_{h}_{kt}")
                g = pt_grp[(qb, h, kt // 4)]
                nc.scalar.activation(g[:, kt % 4, s0:], ps[:, s0:], EXP)
                if causal and 0 <= delta < TB:
                    nc.vector.tensor_tensor(
                        g[:, kt % 4, delta:delta + P],
                        g[:, kt % 4, delta:delta + P],
                        tri_s[:], MULT)
                pt_tiles[(qb, h, kt)] = (g, kt % 4)

            def can_emit():
                if state["emit"] >= len(score_q):
                    return False
                kt = score_q[state["emit"]][2]
                return (kt % 4 != 0
                        or state["alloc"] - state["freed"] < PT_BUFS)

            def pump(credit):
                state["credit"] += credit
                while state["credit"] >= 1.0 and can_emit():
                    state["credit"] -= 1.0
                    emit_score()

            def ensure_scores(qb, h):
                # force-emit any not-yet-pumped score tiles for (qb, h)
                while (state["emit"] < len(score_q)
                       and score_q[state["emit"]][0] * H_PER_CORE
                       + score_q[state["emit"]][1] <= qb * H_PER_CORE + h):
                    assert can_emit(), "pt window deadlock"
                    emit_score()

            def pv_block(qb):
                on_tiles = {}
                ot = otp.tile([P, H_PER_CORE, TB], BF16, tag="ot",
                              name=f"ot_{qb}")

                def transp(h):
                    # transpose+evict head h (norms of h are long done by
                    # the time h+1's chains have been emitted)
                    for qsub in range(4):
                        tp = pss.tile([P, TB], BF16, tag="s",
                                      name=f"tp_{qb}_{h}_{qsub}")
                        nc.tensor.transpose(
                            tp[:, 0:P], on_tiles[(h, qsub)][:], id_s[:])
                        nc.vector.tensor_copy(
                            out=ot[:, h, qsub * P:(qsub + 1) * P],
                            in_=tp[:, 0:P])
                        pump(0.5)

                for h in range(H_PER_CORE):
                    ensure_scores(qb, h)
                    for qsub in range(4):
                        g = 4 * qb + qsub
                        nkt = g + 1 if causal else N_KT
                        pv = pspv.tile([P, HD + 1], FP32, tag="pv",
                                       name=f"pv_{qb}_{h}_{qsub}")
                        for kt in range(nkt):
                            g, sub = pt_tiles[(qb, h, kt)]
                            nc.tensor.matmul(
                                pv[:],
                                g[:, sub, qsub * P:(qsub + 1) * P],
                                v_s[:, h, kt, :],
                                start=kt == 0, stop=kt == nkt - 1)
                        rc = rcp.tile([P, 1], FP32, tag="rc",
                                      name=f"rc_{qb}_{h}_{qsub}")
                        nc.vector.reciprocal_approx_fast(
                            out=rc[:], in_=pv[:, HD:HD + 1])
                        on = onp.tile([P, HD], BF16, tag="on",
                                      name=f"on_{qb}_{h}_{qsub}")
                        nc.vector.tensor_scalar(
                            on[:], pv[:, 0:HD], rc[:], None, MULT)
                        on_tiles[(h, qsub)] = on
                        pump(1.5)
                    state["freed"] += n_kt_q[qb] // 4
                    if h > 0:
                        transp(h - 1)
                transp(H_PER_CORE - 1)
                return ot

            def op_chain(qb, tt, nb):
                row0 = qb * TB + tt * P
                po = pso.tile([P, TB], FP32, tag="po",
                              name=f"po_{qb}_{tt}_{nb}")
                for h in range(H_PER_CORE):
                    nc.tensor.matmul(
                        po[:],
                        ot_tiles[qb][:, h, tt * P:(tt + 1) * P],
                        wo_s[:, h, nb * TB:(nb + 1) * TB],
                        start=h == 0, stop=h == H_PER_CORE - 1)
                ob = obp.tile([P, TB], FP32, tag="ob",
                              name=f"ob_{qb}_{tt}_{nb}")
                # last block: exp is drained, so ACT is free to
                # take half the evictions off the vector engine
                if qb == N_TB - 1 and nb % 2 == 1:
                    nc.scalar.copy(out=ob[:], in_=po[:])
                else:
                    nc.vector.tensor_copy(out=ob[:], in_=po[:])
                nc.sync.dma_start(
                    out_d[row0:row0 + P, nb * TB:(nb + 1) * TB],
                    ob[:])
                pump(1.0)

            # pass-B schedule: V(tb) interleaved with the previous block's
            # output projection (psout evictions get slack under V chains),
            # then PV(tb); the final OP(3) runs at the end.
            ot_tiles = {}
            dma_x(0)  # pass-B x reload, prefetched while pass A drains
            for tb in range(N_TB):
                for tt in range(4):
                    ps = psqk.tile([P, DC], FP32, tag="qk",
                                   name=f"psv_{tb}_{tt}")
                    for kt in range(N_KT):
                        nc.tensor.matmul(
                            ps[:],
                            xh[(tb, kt // 8)][:, kt % 8,
                                              tt * P:(tt + 1) * P],
                            wv_s[:, kt, :],
                            start=kt == 0, stop=kt == N_KT - 1)
                        pump(0.55)
                    if tt == 0 and tb + 1 < N_TB:
                        dma_x(tb + 1)
                    for h in range(H_PER_CORE):
                        nc.vector.tensor_copy(
                            out=v_s[:, h, tb * 4 + tt, 0:HD],
                            in_=ps[:, h * HD:(h + 1) * HD])
                    if tb > 0:
                        for nb in range(4):
                            op_chain(tb - 1, tt, nb)
                # attention for query block tb (keys 0..(tb+1)*512 ready)
                ot_tiles[tb] = pv_block(tb)
            for tt in range(4):
                for nb in range(4):
                    op_chain(N_TB - 1, tt, nb)

    nc.compile()
    return nc


_BASS_CACHE = {}


def _to_bf16(a):
    return np.ascontiguousarray(a).astype(ml_dtypes.bfloat16)


def kernel(x, w_q, w_k, w_v, w_o, causal):
    global LAST_RESULTS
    x = np.asarray(x, dtype=np.float32)
    w_q = np.asarray(w_q, dtype=np.float32)
    w_k = np.asarray(w_k, dtype=np.float32)
    w_v = np.asarray(w_v, dtype=np.float32)
    w_o = np.asarray(w_o, dtype=np.float32)
    is_causal = bool(int(causal))

    if is_causal not in _BASS_CACHE:
        _BASS_CACHE[is_causal] = build_bass(is_causal)
    nc = _BASS_CACHE[is_causal]

    scale = np.float32(1.0 / np.sqrt(HD))
    rr = np.arange(P)[:, None]
    cc = np.arange(P)[None, :]
    tri = (rr <= cc).astype(np.float32)       # keep k<=q in [k,q] diag block
    ident = np.eye(P, dtype=np.float32)

    xT = [np.ascontiguousarray(x[b].T) for b in range(B)]
    in_maps = []
    for c in range(8):
        b, hg = divmod(c, 4)
        cols = slice(hg * DC, (hg + 1) * DC)
        in_maps.append({
            "xT": _to_bf16(xT[b]),
            "wqT": _to_bf16(w_q[cols, :].T * scale),
            "wkT": _to_bf16(w_k[cols, :].T),
            "wvT": _to_bf16(w_v[cols, :].T),
            "woT": _to_bf16(w_o[:, cols].T),
            "tri": _to_bf16(tri),
            "ident": _to_bf16(ident),
        })

    trace = bool(os.environ.get("KERNEL_TRACE"))
    try:
        res = run_bass_kernel_spmd(nc, in_maps, list(range(8)), trace=trace)
    except Exception:
        if not trace:
            raise
        res = run_bass_kernel_spmd(nc, in_maps, list(range(8)), trace=False)
    LAST_RESULTS = res

    out = np.zeros((B, S, D), dtype=np.float32)
    for c in range(8):
        b = c // 4
        out[b] += res.results[c]["out"]
    return out


# revision 40
# speedup vs baseline: 1.0492x; 1.0492x over previous
"""Trainium2 Bass kernel for CustomFlashAttention (B=2, S=2048, D=2048, H=16).

Sharding over 8 NeuronCores: core c handles batch b=c//4 and head-group
hg=c%4 (4 heads of 128 dims = feature cols [hg*512,(hg+1)*512)).
Per core: QKV projections for its cols, causal flash attention for its 4
heads, partial output projection; host sums the 4 partials per batch.

v2 design (bf16): all matmul operands bf16 (fp32 PSUM accumulation),
which enables fast-weight-load and halves DMA/SBUF. Attention uses
natural-orientation PV with a ones-column appended to V so each PV
accumulation also produces the softmax row-sum in PSUM column 128 (no
separate ones-matmul). Per-q normalization is a per-partition
tensor_scalar multiply; the normalized O tile is transposed on the PE
(identity matmul) to feed the output projection. Scores stay in the
transposed [k, q] orientation with exact 128-granular causal trimming;
the diagonal 128x128 block is masked by a 0/1 triangular multiply after
exp (scores are bounded ~|s|<7 so unshifted exp is safe).

Schedule: pass A emits Q+K projections only (pure PE). Pass B emits V
projections with the full attention stream interleaved at matmul
granularity, pacing scores so the Activation engine's exp (the 2nd
busiest resource) always runs ahead of PV consumption. PSUM uses exactly
8 banks (tags qk/s/pv/po x2); the score/exp bf16 tile pool also recycles
the wq/wk weight buffers (same tag) to stay inside SBUF.
"""

import os
import numpy as np
import ml_dtypes

import concourse.bacc as bacc
import concourse.mybir as mybir
import concourse.tile as tile
from concourse.bass_utils import run_bass_kernel_spmd

B = 2
S = 2048
D = 2048
H_PER_CORE = 4
DC = 512          # feature cols per core (4 heads * 128)
HD = 128          # head dim
P = 128
TB = 512          # token block
N_TB = S // TB    # 4
N_KT = S // P     # 16 (128-wide k/token tiles)
FP32 = mybir.dt.float32
BF16 = mybir.dt.bfloat16
EXP = mybir.ActivationFunctionType.Exp
MULT = mybir.AluOpType.mult

PT_BUFS = 13      # rotation groups (4 kt-tiles each) for wq/wk + score tiles

LAST_RESULTS = None  # BassKernelResults from the most recent run (for test.py)


def build_bass(causal: bool):
    nc = bacc.Bacc(None, target_bir_lowering=False, debug=False)

    xT_d = nc.dram_tensor("xT", [D, S], BF16, kind="ExternalInput")
    wqT_d = nc.dram_tensor("wqT", [D, DC], BF16, kind="ExternalInput")
    wkT_d = nc.dram_tensor("wkT", [D, DC], BF16, kind="ExternalInput")
    wvT_d = nc.dram_tensor("wvT", [D, DC], BF16, kind="ExternalInput")
    woT_d = nc.dram_tensor("woT", [DC, D], BF16, kind="ExternalInput")
    tri_d = nc.dram_tensor("tri", [P, P], BF16, kind="ExternalInput")
    id_d = nc.dram_tensor("ident", [P, P], BF16, kind="ExternalInput")
    out_d = nc.dram_tensor("out", [S, D], FP32, kind="ExternalOutput")

    x_r = xT_d.rearrange("(ko p) t -> p ko t", p=P)     # [128, 16, 2048]
    wq_r = wqT_d.rearrange("(ko p) m -> p ko m", p=P)   # [128, 16, 512]
    wk_r = wkT_d.rearrange("(ko p) m -> p ko m", p=P)
    wv_r = wvT_d.rearrange("(ko p) m -> p ko m", p=P)
    wo_r = woT_d.rearrange("(h p) n -> p h n", p=P)     # [128, 4, 2048]

    with tile.TileContext(nc) as tc:
        with tc.tile_pool(name="persist", bufs=1) as persist, \
             tc.tile_pool(name="wp", bufs=PT_BUFS) as wp, \
             tc.tile_pool(name="xp", bufs=4) as xp, \
             tc.tile_pool(name="onp", bufs=16) as onp, \
             tc.tile_pool(name="rcp", bufs=4) as rcp, \
             tc.tile_pool(name="otp", bufs=2) as otp, \
             tc.tile_pool(name="obp", bufs=3) as obp, \
             tc.tile_pool(name="psqk", bufs=2, space="PSUM") as psqk, \
             tc.tile_pool(name="pss", bufs=2, space="PSUM") as pss, \
             tc.tile_pool(name="pspv", bufs=2, space="PSUM") as pspv, \
             tc.tile_pool(name="pso", bufs=2, space="PSUM") as pso:

            # ---- persistent tensors ----
            qt_s = persist.tile([P, H_PER_CORE, S], BF16)   # QT [d, h, tok]
            kt_s = persist.tile([P, H_PER_CORE, S], BF16)   # KT [d, h, tok]
            # V natural + ones column: [tok%128, h, tok//128, 129]
            v_s = persist.tile([P, H_PER_CORE, N_KT, HD + 1], BF16)
            wv_s = persist.tile([P, N_KT, DC], BF16)
            wo_s = persist.tile([P, H_PER_CORE, D], BF16)
            tri_s = persist.tile([P, P], BF16)
            id_s = persist.tile([P, P], BF16)

            # ---- weight/x prefetch. wq/wk live in 4-kt chunks that share
            # the rotation tag (and 4KB slot) with the score/exp tile
            # groups, so their SBUF space is recycled by the attention
            # stream and the whole prefetch is 8 DMA issues. ----
            wq_c = [wp.tile([P, 4, DC], BF16, tag="p", name=f"wq{c}")
                    for c in range(4)]
            wk_c = [wp.tile([P, 4, DC], BF16, tag="p", name=f"wk{c}")
                    for c in range(4)]

            xh = {}  # (tb, half) -> x tile [128 d, 8 kt, 512 tok]

            def dma_x(tb):
                for half in range(2):
                    t = xp.tile([P, 8, TB], BF16, tag="x",
                                name=f"x{tb}_{half}")
                    nc.sync.dma_start(
                        t[:], x_r[:, half * 8:half * 8 + 8,
                                  tb * TB:(tb + 1) * TB])
                    xh[(tb, half)] = t

            def xt(tb, kt):
                return xh[(tb, kt // 8)][:, kt % 8, :]

            # prefetch: the first 4 kt of x and wq land as small per-kt
            # DMAs (fast completion keeps pace with the cold-clock first
            # chain), the rest as chunks; the two DMA queues run in
            # parallel (x on sync, weights on scalar)
            xh0 = xp.tile([P, 8, TB], BF16, tag="x", name="x0_0")
            xh1 = xp.tile([P, 8, TB], BF16, tag="x", name="x0_1")
            xh[(0, 0)], xh[(0, 1)] = xh0, xh1
            for kt in range(4):
                wq_q = nc.scalar if kt % 2 == 0 else nc.sync
                x_q = nc.sync if kt % 2 == 0 else nc.scalar
                wq_q.dma_start(wq_c[0][:, kt, :], wq_r[:, kt, :])
                x_q.dma_start(xh0[:, kt, :], x_r[:, kt, 0:TB])
            nc.sync.dma_start(xh0[:, 4:8, :], x_r[:, 4:8, 0:TB])
            nc.sync.dma_start(xh1[:], x_r[:, 8:16, 0:TB])
            for c in range(1, 4):
                nc.scalar.dma_start(wq_c[c][:], wq_r[:, 4 * c:4 * c + 4, :])
            for c in range(4):
                nc.scalar.dma_start(wk_c[c][:], wk_r[:, 4 * c:4 * c + 4, :])
            nc.scalar.dma_start(tri_s[:], tri_d[:])
            nc.scalar.dma_start(id_s[:], id_d[:])
            nc.scalar.dma_start(wv_s[:], wv_r[:])
            nc.scalar.dma_start(wo_s[:], wo_r[:])
            # ones columns of v_s (memset everything; V evicts overwrite data)
            nc.gpsimd.memset(v_s[:], 1.0)

            # warm-up matmuls on scratch data while the first DMAs land:
            # keeps the PE busy from ~2us so its clock is fully ramped
            # when the real projection chains start
            warm = persist.tile([P, TB], BF16, tag="warm")
            nc.vector.memset(warm[:], 0.0)
            for i in range(3):
                wps = pss.tile([P, TB], FP32, tag="s", name=f"warm{i}")
                nc.tensor.matmul(wps[:], warm[:, 0:P], warm[:],
                                 start=True, stop=True)

            # ---- pass A: Q and K projections (transposed layouts) ----
            for tb in range(N_TB):
                for which, w_c, dst in (("q", wq_c, qt_s), ("k", wk_c, kt_s)):
                    for h in range(H_PER_CORE):
                        ps = psqk.tile([P, TB], FP32, tag="qk",
                                       name=f"ps{which}_{tb}_{h}")
                        for kt in range(N_KT):
                            nc.tensor.matmul(
                                ps[:],
                                w_c[kt // 4][:, kt % 4,
                                             h * HD:(h + 1) * HD],
                                xt(tb, kt),
                                start=kt == 0, stop=kt == N_KT - 1)
                        nc.vector.tensor_copy(
                            out=dst[:, h, tb * TB:(tb + 1) * TB], in_=ps[:])
                        if which == "q" and h == 0 and tb + 1 < N_TB:
                            dma_x(tb + 1)

            # ---- pass B: V projections + interleaved attention ----
            n_kt_q = [4 * qb + 4 if causal else N_KT for qb in range(N_TB)]
            pt_tiles = {}
            pt_grp = {}

            # score/exp emission machinery, paced by a rotation window
            # counted in 4-kt tile groups (wq/wk occupy the first 8 slots)
            score_q = [(qb, h, kt)
                       for qb in range(N_TB)
                       for h in range(H_PER_CORE)
                       for kt in range(n_kt_q[qb])]
            state = {"emit": 0, "alloc": 8, "freed": 8, "credit": 0.0}

            def emit_score():
                qb, h, kt = score_q[state["emit"]]
                state["emit"] += 1
                delta = (kt - 4 * qb) * P if causal else -1
                s0 = max(0, min(delta, TB - P)) if causal else 0
                ps = pss.tile([P, TB], FP32, tag="s", name=f"s_{qb}_{h}_{kt}")
                nc.tensor.matmul(
                    ps[:, s0:],
                    kt_s[:, h, kt * P:(kt + 1) * P],
                    qt_s[:, h, qb * TB + s0:(qb + 1) * TB],
                    start=True, stop=True)
                if kt % 4 == 0:
                    state["alloc"] += 1
                    pt_grp[(qb, h, kt // 4)] = wp.tile(
                        [P, 4, TB], BF16, tag="p", name=f"p_{qb}_{h}_{kt}")
                g = pt_grp[(qb, h, kt // 4)]
                nc.scalar.activation(g[:, kt % 4, s0:], ps[:, s0:], EXP)
                if causal and 0 <= delta < TB:
                    nc.vector.tensor_tensor(
                        g[:, kt % 4, delta:delta + P],
                        g[:, kt % 4, delta:delta + P],
                        tri_s[:], MULT)
                pt_tiles[(qb, h, kt)] = (g, kt % 4)

            def can_emit():
                if state["emit"] >= len(score_q):
                    return False
                kt = score_q[state["emit"]][2]
                return (kt % 4 != 0
                        or state["alloc"] - state["freed"] < PT_BUFS)

            def pump(credit):
                state["credit"] += credit
                while state["credit"] >= 1.0 and can_emit():
                    state["credit"] -= 1.0
                    emit_score()

            def ensure_scores(qb, h):
                # force-emit any not-yet-pumped score tiles for (qb, h)
                while (state["emit"] < len(score_q)
                       and score_q[state["emit"]][0] * H_PER_CORE
                       + score_q[state["emit"]][1] <= qb * H_PER_CORE + h):
                    assert can_emit(), "pt window deadlock"
                    emit_score()

            def pv_block(qb):
                on_tiles = {}
                ot = otp.tile([P, H_PER_CORE, TB], BF16, tag="ot",
                              name=f"ot_{qb}")

                def transp(h):
                    # transpose+evict head h (norms of h are long done by
                    # the time h+1's chains have been emitted)
                    for qsub in range(4):
                        tp = pss.tile([P, TB], BF16, tag="s",
                                      name=f"tp_{qb}_{h}_{qsub}")
                        nc.tensor.transpose(
                            tp[:, 0:P], on_tiles[(h, qsub)][:], id_s[:])
                        nc.vector.tensor_copy(
                            out=ot[:, h, qsub * P:(qsub + 1) * P],
                            in_=tp[:, 0:P])
                        pump(0.5)

                for h in range(H_PER_CORE):
                    ensure_scores(qb, h)
                    for qsub in range(4):
                        g = 4 * qb + qsub
                        nkt = g + 1 if causal else N_KT
                        pv = pspv.tile([P, HD + 1], FP32, tag="pv",
                                       name=f"pv_{qb}_{h}_{qsub}")
                        for kt in range(nkt):
                            g, sub = pt_tiles[(qb, h, kt)]
                            nc.tensor.matmul(
                                pv[:],
                                g[:, sub, qsub * P:(qsub + 1) * P],
                                v_s[:, h, kt, :],
                                start=kt == 0, stop=kt == nkt - 1)
                        rc = rcp.tile([P, 1], FP32, tag="rc",
                                      name=f"rc_{qb}_{h}_{qsub}")
                        nc.vector.reciprocal_approx_fast(
                            out=rc[:], in_=pv[:, HD:HD + 1])
                        on = onp.tile([P, HD], BF16, tag="on",
                                      name=f"on_{qb}_{h}_{qsub}")
                        nc.vector.tensor_scalar(
                            on[:], pv[:, 0:HD], rc[:], None, MULT)
                        on_tiles[(h, qsub)] = on
                        pump(1.5)
                    state["freed"] += n_kt_q[qb] // 4
                    if h > 0:
                        transp(h - 1)
                transp(H_PER_CORE - 1)
                return ot

            def op_chain(qb, tt, nb):
                row0 = qb * TB + tt * P
                po = pso.tile([P, TB], FP32, tag="po",
                              name=f"po_{qb}_{tt}_{nb}")
                for h in range(H_PER_CORE):
                    nc.tensor.matmul(
                        po[:],
                        ot_tiles[qb][:, h, tt * P:(tt + 1) * P],
                        wo_s[:, h, nb * TB:(nb + 1) * TB],
                        start=h == 0, stop=h == H_PER_CORE - 1)
                ob = obp.tile([P, TB], FP32, tag="ob",
                              name=f"ob_{qb}_{tt}_{nb}")
                # last block: exp is drained, so ACT is free to
                # take half the evictions off the vector engine
                if qb == N_TB - 1 and nb % 2 == 1:
                    nc.scalar.copy(out=ob[:], in_=po[:])
                else:
                    nc.vector.tensor_copy(out=ob[:], in_=po[:])
                nc.sync.dma_start(
                    out_d[row0:row0 + P, nb * TB:(nb + 1) * TB],
                    ob[:])
                pump(1.0)

            # pass-B schedule: V(tb) interleaved with the previous block's
            # output projection (psout evictions get slack under V chains),
            # then PV(tb); the final OP(3) runs at the end.
            ot_tiles = {}
            dma_x(0)  # pass-B x reload, prefetched while pass A drains
            for tb in range(N_TB):
                for tt in range(4):
                    ps = psqk.tile([P, DC], FP32, tag="qk",
                                   name=f"psv_{tb}_{tt}")
                    for kt in range(N_KT):
                        nc.tensor.matmul(
                            ps[:],
                            xh[(tb, kt // 8)][:, kt % 8,
                                              tt * P:(tt + 1) * P],
                            wv_s[:, kt, :],
                            start=kt == 0, stop=kt == N_KT - 1)
                        pump(0.55)
                    if tt == 0 and tb + 1 < N_TB:
                        dma_x(tb + 1)
                    for h in range(H_PER_CORE):
                        nc.vector.tensor_copy(
                            out=v_s[:, h, tb * 4 + tt, 0:HD],
                            in_=ps[:, h * HD:(h + 1) * HD])
                # attention for query block tb (keys 0..(tb+1)*512 ready)
                ot_tiles[tb] = pv_block(tb)
                for tt in range(4):
                    for nb in range(4):
                        op_chain(tb, tt, nb)

    nc.compile()
    return nc


_BASS_CACHE = {}


def _to_bf16(a):
    return np.ascontiguousarray(a).astype(ml_dtypes.bfloat16)


def kernel(x, w_q, w_k, w_v, w_o, causal):
    global LAST_RESULTS
    x = np.asarray(x, dtype=np.float32)
    w_q = np.asarray(w_q, dtype=np.float32)
    w_k = np.asarray(w_k, dtype=np.float32)
    w_v = np.asarray(w_v, dtype=np.float32)
    w_o = np.asarray(w_o, dtype=np.float32)
    is_causal = bool(int(causal))

    if is_causal not in _BASS_CACHE:
        _BASS_CACHE[is_causal] = build_bass(is_causal)
    nc = _BASS_CACHE[is_causal]

    scale = np.float32(1.0 / np.sqrt(HD))
    rr = np.arange(P)[:, None]
    cc = np.arange(P)[None, :]
    tri = (rr <= cc).astype(np.float32)       # keep k<=q in [k,q] diag block
    ident = np.eye(P, dtype=np.float32)

    xT = [np.ascontiguousarray(x[b].T) for b in range(B)]
    in_maps = []
    for c in range(8):
        b, hg = divmod(c, 4)
        cols = slice(hg * DC, (hg + 1) * DC)
        in_maps.append({
            "xT": _to_bf16(xT[b]),
            "wqT": _to_bf16(w_q[cols, :].T * scale),
            "wkT": _to_bf16(w_k[cols, :].T),
            "wvT": _to_bf16(w_v[cols, :].T),
            "woT": _to_bf16(w_o[:, cols].T),
            "tri": _to_bf16(tri),
            "ident": _to_bf16(ident),
        })

    trace = bool(os.environ.get("KERNEL_TRACE"))
    try:
        res = run_bass_kernel_spmd(nc, in_maps, list(range(8)), trace=trace)
    except Exception:
        if not trace:
            raise
        res = run_bass_kernel_spmd(nc, in_maps, list(range(8)), trace=False)
    LAST_RESULTS = res

    out = np.zeros((B, S, D), dtype=np.float32)
    for c in range(8):
        b = c // 4
        out[b] += res.results[c]["out"]
    return out
